# revision 1
# baseline (speedup 1.0000x reference)
"""Trainium2 Bass kernel for nn_ComplexQuantumLayer (10-qubit circuit, batch 2048).

Math: the circuit after the RX AngleEmbedding is a fixed unitary U (depends only
on `weights`), and the embedded state is a Kronecker product
  psi0[b] = (-i)^popcount(j) * m[b, j],   m[b] = kron_q [cos(x_bq/2), sin(x_bq/2)].
Folding the phase into W = diag(phase) @ U^T gives  psi = m @ W  with m REAL.
Per sample the device does two real (1024,1024) matvecs, |psi|^2, and the ten
PauliZ sums as one more matmul against a +/-1 mask matrix.

Device pipeline (per core, 256 samples, fp16 operands / fp32 accumulation):
  1. Kronecker tree -> m (batch, amp) fp32, last level written fp16
  2. PE transposes  -> mT (amp, batch) fp16
  3. psiT[nchunk]   = Wchunk.T @ mT      (PE, fp16, fp32 PSUM)
  4. p[nchunk]      = Re^2 + Im^2        (ACT squares + DVE add, fp16)
  5. zsum[b, q]     = sum_n p[n, b] z_q[n]  (PE, lhsT = p, rhs = Z-mask chunk)
W is shipped in a (8, 128, 16, 128) tiled layout (4KB DMA lines) so each DMA
delivers everything one output-chunk iteration needs; Z-masks ride along in the
same tensor (slot 16 of an extended 17-slot axis).

Sharding: pure data parallel - batch 2048 split as 256 rows per each of the
8 cores; W (fp16, ~4.3MB) replicated per core.
"""

import numpy as np

import concourse.bass as bass
import concourse.bacc as bacc
import concourse.mybir as mybir
from concourse.bass_utils import run_bass_kernel_spmd
from concourse.masks import make_identity
from concourse.tile import TileContext

NQ = 10
DIM = 1 << NQ          # 1024
BATCH = 2048
NCORES = 8
BPC = BATCH // NCORES  # 256 rows per core
P = 128                # partitions
BT = BPC // P          # batch tiles per core = 2
KC = DIM // P          # in-amp chunks = 8
JC = DIM // P          # out-amp chunks = 8

F32 = mybir.dt.float32
F16 = mybir.dt.float16

LAST_RESULT = None  # BassKernelResults of the most recent run (for test harness)


# ----------------------------------------------------------------------------
# Host-side preprocessing: circuit unitary from weights (numpy, ~2s)
# ----------------------------------------------------------------------------

def _build_circuit_matrix(weights: np.ndarray, dtype=np.complex128) -> np.ndarray:
    """M = U^T: the reference circuit (post-embedding) applied to identity rows."""
    w = weights.astype(np.float64)
    state = np.eye(DIM, dtype=dtype)

    def apply_1q(state, g, q):
        s = state.reshape(DIM, 1 << q, 2, -1)
        s0 = s[:, :, 0, :].copy()
        s1 = s[:, :, 1, :].copy()
        s[:, :, 0, :] = g[0, 0] * s0 + g[0, 1] * s1
        s[:, :, 1, :] = g[1, 0] * s0 + g[1, 1] * s1
        return state

    def apply_2q(state, g4, q1, q2):
        g = g4.reshape(2, 2, 2, 2)
        if q1 > q2:
            g = np.transpose(g, (1, 0, 3, 2))
            q1, q2 = q2, q1
        A = 1 << q1
        M = 1 << (q2 - q1 - 1)
        s = state.reshape(DIM, A, 2, M, 2, -1)
        blocks = [s[:, :, c, :, d, :].copy() for c in (0, 1) for d in (0, 1)]
        for a in (0, 1):
            for b in (0, 1):
                acc = None
                for c in (0, 1):
                    for d in (0, 1):
                        coef = g[a, b, c, d]
                        if coef == 0:
                            continue
                        term = coef * blocks[2 * c + d]
                        acc = term if acc is None else acc + term
                s[:, :, a, :, b, :] = 0 if acc is None else acc
        return state

    def rot_matrix(phi, theta, omega):
        ct, st = np.cos(theta / 2), np.sin(theta / 2)
        return np.array(
            [[np.exp(-0.5j * (phi + omega)) * ct, -np.exp(0.5j * (phi - omega)) * st],
             [np.exp(-0.5j * (phi - omega)) * st, np.exp(0.5j * (phi + omega)) * ct]]
        )

    CNOT = np.array([[1, 0, 0, 0], [0, 1, 0, 0], [0, 0, 0, 1], [0, 0, 1, 0]], dtype)
    I4 = np.eye(4, dtype=dtype)
    XX = np.array([[0, 0, 0, 1], [0, 0, 1, 0], [0, 1, 0, 0], [1, 0, 0, 0]], dtype)
    YY = np.array([[0, 0, 0, -1], [0, 0, 1, 0], [0, 1, 0, 0], [-1, 0, 0, 0]], dtype)

    n_layers = w.shape[0]
    for l in range(n_layers):
        wl = w[l]
        for q in range(NQ):
            state = apply_1q(state, rot_matrix(*wl[q]), q)
        for q in range(NQ):
            state = apply_2q(state, CNOT, q, (q + 1) % NQ)
        c, s_ = np.cos(wl[0, 0] / 2), np.sin(wl[0, 0] / 2)
        state = apply_2q(state, c * I4 + (-1j * s_) * XX, 0, 1)
        c, s_ = np.cos(wl[0, 1] / 2), np.sin(wl[0, 1] / 2)
        state = apply_2q(state, c * I4 + (-1j * s_) * YY, 1, 2)
        e, ec = np.exp(-0.5j * wl[0, 2]), np.exp(0.5j * wl[0, 2])
        state = apply_2q(state, np.diag(np.array([e, ec, ec, e])), 2, 3)
    return state


def _host_prepare(x: np.ndarray, weights: np.ndarray):
    M = _build_circuit_matrix(weights)
    pc = np.array([bin(k).count("1") for k in range(DIM)])
    W = ((-1j) ** pc)[:, None] * M
    wr = W.real.astype(np.float16)   # (1024, 1024) [k, n]
    wi = W.imag.astype(np.float16)

    # wt[j, p, s, c]: j = out-amp chunk, p = in-amp within chunk,
    # s in 0..7 -> (in-chunk ko=s, real), 8..15 -> (ko=s-8, imag),
    # s = 16 -> Z-mask rows: wt[j, p, 16, q] = 1 - 2*bit_q(j*128 + p)
    wr4 = wr.reshape(KC, P, JC, P).transpose(2, 1, 0, 3)  # [j, p, ko, c]
    wi4 = wi.reshape(KC, P, JC, P).transpose(2, 1, 0, 3)
    wt = np.zeros((JC, P, 17, P), dtype=np.float16)
    wt[:, :, 0:8, :] = wr4
    wt[:, :, 8:16, :] = wi4
    n = np.arange(DIM)
    zm = (1 - 2 * ((n[:, None] >> (NQ - 1 - np.arange(NQ))[None, :]) & 1)).astype(
        np.float16
    )  # (1024, 10)
    wt[:, :, 16, :NQ] = zm.reshape(JC, P, NQ)
    wt = np.ascontiguousarray(wt)

    xd = x.astype(np.float64)
    cs = np.concatenate([np.cos(xd / 2), np.sin(xd / 2)], axis=1).astype(np.float32)
    return cs, wt


# ----------------------------------------------------------------------------
# Bass kernel (per-core program; SPMD across 8 cores)
# ----------------------------------------------------------------------------

def _build_bass() -> bass.Bass:
    nc = bacc.Bacc(trn_type="TRN2")

    cs_d = nc.dram_tensor("cs", (BPC, 2 * NQ), F32, kind="ExternalInput")
    wt_d = nc.dram_tensor("wt", (JC, P, 17, P), F16, kind="ExternalInput")
    out_d = nc.dram_tensor("out", (BPC, NQ), F32, kind="ExternalOutput")

    with TileContext(nc) as tc:
        with (
            tc.tile_pool(name="wpool", bufs=1) as w_pool,
            tc.tile_pool(name="work", bufs=1) as work_pool,
            tc.tile_pool(name="scr", bufs=2) as scr_pool,
            tc.tile_pool(name="tpsum", bufs=2, space="PSUM") as tpsum,
            tc.tile_pool(name="mpsum", bufs=4, space="PSUM") as mpsum,
            tc.tile_pool(name="zpsum", bufs=1, space="PSUM") as zpsum,
        ):
            # ---- W DMAs first: 4 partition-sliced DMAs per out-chunk j, spread
            # across 4 issue queues so descriptors parallelize and chunk j=0
            # completes early (matmuls start as soon as their chunk lands).
            w_sb = []
            for j in range(JC):
                t = w_pool.tile([P, 17, P], F16, name=f"w_{j}")
                eng = nc.sync if j % 2 == 0 else nc.scalar
                eng.dma_start(t, wt_d[j])
                w_sb.append(t)

            # ---- cos/sin inputs
            cs_sb = []
            for bt in range(BT):
                t = work_pool.tile([P, 2 * NQ], F32, name=f"cs_{bt}")
                nc.sync.dma_start(t, cs_d[bt * P:(bt + 1) * P, :])
                cs_sb.append(t)

            identity = work_pool.tile([P, P], F16, name="identity")
            make_identity(nc, identity)

            # ---- Kronecker trees (fp32, final level -> fp16): bt0 DVE, bt1 ACT
            mb_sb = []
            for bt in range(BT):
                csb = cs_sb[bt]
                m = work_pool.tile([P, DIM // 2], F32, name=f"m_{bt}")
                mb = work_pool.tile([P, DIM], F16, name=f"mb_{bt}")
                if bt == 0:
                    cp = nc.vector.tensor_copy
                    mul = nc.vector.tensor_scalar_mul
                else:
                    cp = nc.scalar.copy
                    mul = lambda out, in0, scalar1: nc.scalar.mul(out, in0, scalar1)
                cp(m[:, 0:1], csb[:, 9:10])
                cp(m[:, 1:2], csb[:, 19:20])
                L = 2
                for q in range(8, 0, -1):
                    mul(m[:, L:2 * L], m[:, 0:L], csb[:, 10 + q:11 + q])
                    mul(m[:, 0:L], m[:, 0:L], csb[:, q:q + 1])
                    L *= 2
                # final level (q=0) writes fp16
                mul(mb[:, L:2 * L], m[:, 0:L], csb[:, 10:11])
                mul(mb[:, 0:L], m[:, 0:L], csb[:, 0:1])
                mb_sb.append(mb)

            # ---- transposes: mt[:, k, bt*128:] = (amp chunk k, batch) fp16
            mt = work_pool.tile([P, KC, BPC], F16, name="mt")
            for bt in range(BT):
                for k in range(KC):
                    tp = tpsum.tile([P, P], F16, name="tp", tag="tp")
                    nc.tensor.transpose(tp, mb_sb[bt][:, k * P:(k + 1) * P], identity)
                    nc.vector.tensor_copy(mt[:, k, bt * P:(bt + 1) * P], tp)

            # ---- per out-chunk: matmuls, |psi|^2, Z-mask matmul
            zp = [zpsum.tile([P, NQ], F32, name=f"zp_{bt}") for bt in range(BT)]
            p_sb = work_pool.tile([P, JC, BPC], F16, name="p_sb")
            for j in range(JC):
                ps_r = mpsum.tile([P, BPC], F32, name="psr", tag="mmps")
                ps_i = mpsum.tile([P, BPC], F32, name="psi", tag="mmps")
                for k in range(KC):
                    nc.tensor.matmul(
                        ps_r, lhsT=w_sb[j][:, k, :], rhs=mt[:, k, :],
                        start=(k == 0), stop=(k == KC - 1),
                    )
                for k in range(KC):
                    nc.tensor.matmul(
                        ps_i, lhsT=w_sb[j][:, 8 + k, :], rhs=mt[:, k, :],
                        start=(k == 0), stop=(k == KC - 1),
                    )
                sq_r = scr_pool.tile([P, BPC], F32, name="sq_r", tag="sqr")
                sq_i = scr_pool.tile([P, BPC], F32, name="sq_i", tag="sqi")
                nc.scalar.square(sq_r, ps_r)
                nc.scalar.square(sq_i, ps_i)
                nc.vector.tensor_add(p_sb[:, j, :], sq_r, sq_i)
                for bt in range(BT):
                    nc.tensor.matmul(
                        zp[bt],
                        lhsT=p_sb[:, j, bt * P:(bt + 1) * P],
                        rhs=w_sb[j][:, 16, 0:NQ],
                        start=(j == 0), stop=(j == JC - 1),
                        skip_group_check=True,
                    )

            # ---- out
            for bt in range(BT):
                outsb = work_pool.tile([P, NQ], F32, name=f"o_{bt}")
                nc.scalar.copy(outsb, zp[bt])
                nc.sync.dma_start(out_d[bt * P:(bt + 1) * P, :], outsb)

    nc.finalize()
    return nc


# ----------------------------------------------------------------------------
# Entry point
# ----------------------------------------------------------------------------

def kernel(x: np.ndarray, weights: np.ndarray, _trace: bool = False) -> np.ndarray:
    global LAST_RESULT
    x = np.asarray(x, dtype=np.float32)
    weights = np.asarray(weights, dtype=np.float32)

    cs, wt = _host_prepare(x, weights)

    nc = _build_bass()
    in_maps = [
        {"cs": np.ascontiguousarray(cs[i * BPC:(i + 1) * BPC]), "wt": wt}
        for i in range(NCORES)
    ]
    res = run_bass_kernel_spmd(
        nc, in_maps, core_ids=list(range(NCORES)), trace=_trace
    )
    LAST_RESULT = res
    out = np.concatenate([r["out"] for r in res.results], axis=0)
    return out.astype(np.float32)



# revision 9
# speedup vs baseline: 1.4216x; 1.4216x over previous
"""Trainium2 Bass kernel for nn_ComplexQuantumLayer (10-qubit circuit, batch 2048).

Math: the circuit after the RX AngleEmbedding is a fixed unitary U (depends only
on `weights`), and the embedded state is a Kronecker product
  psi0[b] = (-i)^popcount(j) * m[b, j],   m[b] = kron_q [cos(x_bq/2), sin(x_bq/2)].
Folding the phase into W = diag(phase) @ U^T gives  psi = m @ W  with m REAL.
Per sample the device does two real (1024,1024) matvecs, |psi|^2, and the ten
PauliZ sums as one more matmul against a +/-1 mask matrix.

Device pipeline (per core, 256 samples = 2 partition tiles, fp16 operands):
  1. quad-product Kronecker combine -> m (batch, amp) fp16   [DVE + GpSimd]
  2. 16 PE transposes -> mt (amp, batch) fp16
  3. psiT[jchunk] = Wchunk.T @ mt (8 fp16 matmuls, fp32 PSUM) x2 (re/im)
  4. p = re^2 (ACT) + im^2 (DVE), summed on GpSimd, fp16
  5. zT[10, 256] += zmask_chunk.T @ p (one small PE matmul per chunk)
  6. two PE transposes + copies -> out (256, 10) fp32
The host sends, per sample, 36 quad products of cos/sin (16 for qubits 0-3,
16 for qubits 4-7, 4 for qubits 8-9), so the device tree is 2 levels of
broadcast-AP tensor_tensor multiplies.

Schedule notes: the pp DMA is issued first on sync so the tree starts ~0.5us
into the body; W chunk DMAs are spread over sync/scalar/tensor queues so the
PE matmul stream is never starved; squares/adds ride ACT/DVE/GpSimd so the
PE does only transposes + matmuls.

Sharding: pure data parallel - batch 2048 split as 256 rows per each of the
8 cores; W (fp16, ~4.3MB) replicated per core.
"""

import numpy as np

import concourse.bass as bass
import concourse.bacc as bacc
import concourse.mybir as mybir
from concourse.bass_utils import run_bass_kernel_spmd
from concourse.masks import make_identity
from concourse.tile import TileContext

NQ = 10
DIM = 1 << NQ          # 1024
BATCH = 2048
NCORES = 8
BPC = BATCH // NCORES  # 256 rows per core
P = 128                # partitions
BT = BPC // P          # batch tiles per core = 2
KC = DIM // P          # in-amp chunks = 8
JC = DIM // P          # out-amp chunks = 8

F32 = mybir.dt.float32
F16 = mybir.dt.float16
MUL = mybir.AluOpType.mult
ADD = mybir.AluOpType.add

LAST_RESULT = None  # BassKernelResults of the most recent run (for test harness)


# ----------------------------------------------------------------------------
# Host-side preprocessing: circuit unitary from weights (numpy, ~2s)
# ----------------------------------------------------------------------------

def _build_circuit_matrix(weights: np.ndarray, dtype=np.complex128) -> np.ndarray:
    """M = U^T: the reference circuit (post-embedding) applied to identity rows."""
    w = weights.astype(np.float64)
    state = np.eye(DIM, dtype=dtype)

    def apply_1q(state, g, q):
        s = state.reshape(DIM, 1 << q, 2, -1)
        s0 = s[:, :, 0, :].copy()
        s1 = s[:, :, 1, :].copy()
        s[:, :, 0, :] = g[0, 0] * s0 + g[0, 1] * s1
        s[:, :, 1, :] = g[1, 0] * s0 + g[1, 1] * s1
        return state

    def apply_2q(state, g4, q1, q2):
        g = g4.reshape(2, 2, 2, 2)
        if q1 > q2:
            g = np.transpose(g, (1, 0, 3, 2))
            q1, q2 = q2, q1
        A = 1 << q1
        M = 1 << (q2 - q1 - 1)
        s = state.reshape(DIM, A, 2, M, 2, -1)
        blocks = [s[:, :, c, :, d, :].copy() for c in (0, 1) for d in (0, 1)]
        for a in (0, 1):
            for b in (0, 1):
                acc = None
                for c in (0, 1):
                    for d in (0, 1):
                        coef = g[a, b, c, d]
                        if coef == 0:
                            continue
                        term = coef * blocks[2 * c + d]
                        acc = term if acc is None else acc + term
                s[:, :, a, :, b, :] = 0 if acc is None else acc
        return state

    def rot_matrix(phi, theta, omega):
        ct, st = np.cos(theta / 2), np.sin(theta / 2)
        return np.array(
            [[np.exp(-0.5j * (phi + omega)) * ct, -np.exp(0.5j * (phi - omega)) * st],
             [np.exp(-0.5j * (phi - omega)) * st, np.exp(0.5j * (phi + omega)) * ct]]
        )

    CNOT = np.array([[1, 0, 0, 0], [0, 1, 0, 0], [0, 0, 0, 1], [0, 0, 1, 0]], dtype)
    I4 = np.eye(4, dtype=dtype)
    XX = np.array([[0, 0, 0, 1], [0, 0, 1, 0], [0, 1, 0, 0], [1, 0, 0, 0]], dtype)
    YY = np.array([[0, 0, 0, -1], [0, 0, 1, 0], [0, 1, 0, 0], [-1, 0, 0, 0]], dtype)

    n_layers = w.shape[0]
    for l in range(n_layers):
        wl = w[l]
        for q in range(NQ):
            state = apply_1q(state, rot_matrix(*wl[q]), q)
        for q in range(NQ):
            state = apply_2q(state, CNOT, q, (q + 1) % NQ)
        c, s_ = np.cos(wl[0, 0] / 2), np.sin(wl[0, 0] / 2)
        state = apply_2q(state, c * I4 + (-1j * s_) * XX, 0, 1)
        c, s_ = np.cos(wl[0, 1] / 2), np.sin(wl[0, 1] / 2)
        state = apply_2q(state, c * I4 + (-1j * s_) * YY, 1, 2)
        e, ec = np.exp(-0.5j * wl[0, 2]), np.exp(0.5j * wl[0, 2])
        state = apply_2q(state, np.diag(np.array([e, ec, ec, e])), 2, 3)
    return state


def _host_prepare(x: np.ndarray, weights: np.ndarray):
    M = _build_circuit_matrix(weights)
    pc = np.array([bin(k).count("1") for k in range(DIM)])
    W = ((-1j) ** pc)[:, None] * M
    wr = W.real.astype(np.float16)   # (1024, 1024) [k, n]
    wi = W.imag.astype(np.float16)

    # wt[j, p, s, c]: j = out-amp chunk, p = in-amp within chunk,
    # s in 0..7 -> (in-chunk ko=s, real), 8..15 -> (ko=s-8, imag),
    # s = 16 -> Z-mask rows: wt[j, p, 16, q] = 1 - 2*bit_q(j*128 + p)
    wr4 = wr.reshape(KC, P, JC, P).transpose(2, 1, 0, 3)  # [j, p, ko, c]
    wi4 = wi.reshape(KC, P, JC, P).transpose(2, 1, 0, 3)
    wt = np.zeros((JC, P, 17, P), dtype=np.float16)
    wt[:, :, 0:8, :] = wr4
    wt[:, :, 8:16, :] = wi4
    n = np.arange(DIM)
    zm = (1 - 2 * ((n[:, None] >> (NQ - 1 - np.arange(NQ))[None, :]) & 1)).astype(
        np.float16
    )  # (1024, 10)
    wt[:, :, 16, :NQ] = zm.reshape(JC, P, NQ)
    wt = np.ascontiguousarray(wt)

    # quad products: cols 0:16 = qubits 0-3 (digit = 8b0+4b1+2b2+b3),
    # 16:32 = qubits 4-7, 32:36 = qubits 8-9 (digit = 2b8+b9)
    xd = x.astype(np.float64)
    c = np.cos(xd / 2)
    s = np.sin(xd / 2)
    B = x.shape[0]
    pp = np.empty((B, 36), dtype=np.float32)

    def quad(qs):
        out = np.ones((B, 1))
        for q in qs:
            f = np.stack([c[:, q], s[:, q]], axis=1)
            out = (out[:, :, None] * f[:, None, :]).reshape(B, -1)
        return out

    pp[:, 0:16] = quad((0, 1, 2, 3))
    pp[:, 16:32] = quad((4, 5, 6, 7))
    pp[:, 32:36] = quad((8, 9))
    return pp, wt


# ----------------------------------------------------------------------------
# Bass kernel (per-core program; SPMD across 8 cores)
# ----------------------------------------------------------------------------

def _build_bass() -> bass.Bass:
    nc = bacc.Bacc(trn_type="TRN2")

    pp_d = nc.dram_tensor("pp", (BPC, 36), F32, kind="ExternalInput")
    wt_d = nc.dram_tensor("wt", (JC, P, 17, P), F16, kind="ExternalInput")
    out_d = nc.dram_tensor("out", (BPC, NQ), F32, kind="ExternalOutput")

    with TileContext(nc) as tc:
        with (
            tc.tile_pool(name="wpool", bufs=1) as w_pool,
            tc.tile_pool(name="work", bufs=1) as work_pool,
            tc.tile_pool(name="scr", bufs=2) as scr_pool,
            tc.tile_pool(name="tpsum", bufs=2, space="PSUM") as tpsum,
            tc.tile_pool(name="mpsum", bufs=2, space="PSUM") as mpsum,
            tc.tile_pool(name="zpsum", bufs=1, space="PSUM") as zpsum,
        ):
            # ---- pp DMA first (sync) so the tree can start immediately;
            # W chunk DMAs spread across sync/scalar/tensor issue queues.
            pp_sb = []
            for bt in range(BT):
                t = work_pool.tile([P, 36], F32, name=f"pp_{bt}")
                nc.sync.dma_start(t[:], pp_d[bt * P:(bt + 1) * P, :])
                pp_sb.append(t)

            w_sb = []
            for j in range(JC):
                t = w_pool.tile([P, 17, P], F16, name=f"w_{j}")
                w_sb.append(t)
            issue = [nc.sync, nc.scalar, nc.sync, nc.scalar, nc.sync,
                     nc.scalar, nc.sync, nc.scalar]
            for j in range(JC):
                issue[j].dma_start(w_sb[j][:], wt_d[j])

            identity = work_pool.tile([P, P], F16, name="identity")
            make_identity(nc, identity)

            # ---- 2-level Kronecker combine from quad products -> mb (fp16)
            # l1[p, dB*4 + dC] = ppB[dB] * ppC[dC]          (64 cols, fp32)
            # mb[p, dA*64 + t] = ppA[dA] * l1[t]            (1024 cols, fp16)
            l1_sb = []
            mb_sb = []
            for bt in range(BT):
                l1 = work_pool.tile([P, 64], F32, name=f"l1_{bt}")
                mb = work_pool.tile([P, DIM], F16, name=f"mb_{bt}")
                l1_sb.append(l1)
                mb_sb.append(mb)
            for bt in range(BT):
                pp = pp_sb[bt]
                dv = l1_sb[bt][:, :].rearrange("p (a b) -> p a b", a=16)
                s0 = pp[:, 32:36].unsqueeze(1).to_broadcast((P, 16, 4))
                s1 = pp[:, 16:32].unsqueeze(2).to_broadcast((P, 16, 4))
                nc.vector.tensor_tensor(dv, s1, s0, MUL)
            # level 2: split dA range across DVE (0:10) and GpSimd (10:16)
            for bt in range(BT):
                l1 = l1_sb[bt]
                mb = mb_sb[bt]
                for eng, lo, hi in ((nc.vector, 0, 10), (nc.gpsimd, 10, 16)):
                    n = hi - lo
                    dv = mb[:, lo * 64:hi * 64].rearrange("p (a b) -> p a b", a=n)
                    s0 = l1[:, :].unsqueeze(1).to_broadcast((P, n, 64))
                    s1 = pp_sb[bt][:, lo:hi].unsqueeze(2).to_broadcast((P, n, 64))
                    eng.tensor_tensor(dv, s1, s0, MUL)

            # ---- transposes: mt[:, k, bt*128:] = (amp chunk k, batch) fp16
            mt = work_pool.tile([P, KC, BPC], F16, name="mt")
            for bt in range(BT):
                for k in range(KC):
                    tp = tpsum.tile([P, P], F16, name="tp", tag="tp")
                    nc.tensor.transpose(tp, mb_sb[bt][:, k * P:(k + 1) * P], identity)
                    nc.vector.tensor_copy(mt[:, k, bt * P:(bt + 1) * P], tp)

            # ---- per out-chunk: matmuls, |psi|^2, Z-mask matmul
            zp = zpsum.tile([NQ, BPC], F32, name="zp")
            p_sb = work_pool.tile([P, JC, BPC], F16, name="p_sb")
            for j in range(JC):
                ps_r = mpsum.tile([P, BPC], F32, name="psr", tag="mmps")
                ps_i = mpsum.tile([P, BPC], F32, name="psi", tag="mmps")
                for k in range(KC):
                    nc.tensor.matmul(
                        ps_r, lhsT=w_sb[j][:, k, :], rhs=mt[:, k, :],
                        start=(k == 0), stop=(k == KC - 1),
                    )
                for k in range(KC):
                    nc.tensor.matmul(
                        ps_i, lhsT=w_sb[j][:, 8 + k, :], rhs=mt[:, k, :],
                        start=(k == 0), stop=(k == KC - 1),
                    )
                sq_r = scr_pool.tile([P, BPC], F16, name="sq_r", tag="sqr")
                sq_i = scr_pool.tile([P, BPC], F16, name="sq_i", tag="sqi")
                nc.scalar.square(sq_r, ps_r)
                nc.scalar.square(sq_i, ps_i)
                nc.gpsimd.tensor_tensor(p_sb[:, j, :], sq_r, sq_i, ADD)
                nc.tensor.matmul(
                    zp, lhsT=w_sb[j][:, 16, 0:NQ], rhs=p_sb[:, j, :],
                    start=(j == 0), stop=(j == JC - 1),
                    skip_group_check=True,
                )

            # ---- out: zp is [10, 256]; transpose per batch tile -> (256, 10)
            zsb = work_pool.tile([NQ, BPC], F16, name="zsb")
            nc.vector.tensor_copy(zsb, zp)
            ztr = zpsum.tile([P, BT, NQ], F16, name="ztr")
            for bt in range(BT):
                nc.tensor.matmul(
                    ztr[:, bt, :], lhsT=zsb[:, bt * P:(bt + 1) * P],
                    rhs=identity[0:NQ, 0:NQ], is_transpose=True,
                )
                outsb = work_pool.tile([P, NQ], F32, name=f"o_{bt}")
                nc.scalar.copy(outsb, ztr[:, bt, :])
                nc.sync.dma_start(out_d[bt * P:(bt + 1) * P, :], outsb[:])

    nc.finalize()
    return nc


# ----------------------------------------------------------------------------
# Entry point
# ----------------------------------------------------------------------------

def kernel(x: np.ndarray, weights: np.ndarray, _trace: bool = False) -> np.ndarray:
    global LAST_RESULT
    x = np.asarray(x, dtype=np.float32)
    weights = np.asarray(weights, dtype=np.float32)

    pp, wt = _host_prepare(x, weights)

    nc = _build_bass()
    in_maps = [
        {"pp": np.ascontiguousarray(pp[i * BPC:(i + 1) * BPC]), "wt": wt}
        for i in range(NCORES)
    ]
    res = run_bass_kernel_spmd(
        nc, in_maps, core_ids=list(range(NCORES)), trace=_trace
    )
    LAST_RESULT = res
    out = np.concatenate([r["out"] for r in res.results], axis=0)
    return out.astype(np.float32)
